# revision 51
# baseline (speedup 1.0000x reference)
"""Trainium2 Bass kernel for nn_CategorySpecificInitNet (moe_routing).

kernel(**inputs) takes the FULL unsharded inputs (keys as in
reference.setup_inputs()) and returns the FULL [B, 128] float32 output.

Strategy — expert-parallel with overflow tails:
  - rows are dispatched to cores by category (host-side all-to-all of the
    expert-parallel alternative in the sharding hint): core k's MAIN block
    is 4096 rows (8 tiles x 512) of category k, so the whole main block
    runs ONE decoder with no routing/masking;
  - categories with more than 4096 rows spill their overflow into small
    per-core TAIL tiles (32 rows each, single category per core, possibly
    a different category than the core's main block) with a second decoder
    weight set "B". This keeps per-core work at 4096+32 rows instead of
    padding every core to the max category count — a ~6% tensor saving;
  - the encoder's third linear layer is constant-folded into each
    decoder's first layer on the host (W_f = We3 @ Wd1_k, exact algebra);
  - main tiles run with bf16 MOVING operands (features, inter-layer
    activations) against fp16 stationary weights (10 mantissa bits,
    tf32-grade; the hardware forbids mixing f32/f32r into a 16-bit
    matmul): the PE runs bf16 moving data at one row/cycle like fp32r,
    but the feature DMA halves, which is what lets tile 0's first GEMM
    start ~4.4us into the kernel streaming contraction-chunk-by-chunk
    behind the interleaved we1/features DMAs. The 32-wide tail also runs
    bf16-moving and shares the fp16 weights (encoder) / gets its own fp16
    decoder set "B";
  - relu/copy-out work is spread over the scalar and vector queues in a
    measured assignment (a1: s,s,v,v / a2: s,v / d1: v,v / d2: v,s /
    copy-out: v, with the second-to-last tile's copy-out halved across
    both act engines) — the tile scheduler coalesces activation waits
    into per-queue event semaphores, and the wrong assignment puts a
    just-in-time activation on the critical path of the next stage's
    first matmul (~117ns stall per tile); the last two output stores
    ride the sync queue so the kernel's closing store chain avoids the
    gpsimd queue's ~1us serial descriptor-gen;
  - the biases of this module are structurally zero (setup_inputs zeroes
    every bias), so the default graph variant skips bias loads and applies
    plain relu; a with-bias variant is built if any bias is nonzero;
  - compute is feature-major [features(partitions), rows(free)]; the host
    passes features pre-transposed and inverse-permutes rows on unshard;
  - per-core row tiles of 512, decoder software-pipelined one tile behind
    the encoder ([enc t][d2/d3 t-1][d1 t]);
  - a warm-up chain of dummy 64-wide matmuls keeps the tensor engine
    continuously busy from ~1us so its p-state ramp (half clock for the
    first 3us of any cold start) completes while the first DMAs are still
    in flight, and the real GEMMs run at full clock from the start.

Measured end-to-end error vs the fp32 reference: 4.0e-3 max-rel (gate 2e-2).
Measured HW exec time: 82943 ns (baseline kernel: 94286 ns).
"""
import sys

for _p in ("/opt/trn_rl_repo",):
    if _p not in sys.path:
        sys.path.append(_p)

import numpy as np
import ml_dtypes

import concourse.bass as bass
import concourse.bacc as bacc
import concourse.mybir as mybir
import concourse.tile as tile
from concourse import bass_utils

FR = mybir.dt.float32r
F32 = mybir.dt.float32
BF = mybir.dt.bfloat16
FP16 = mybir.dt.float16
BF_NP = ml_dtypes.bfloat16
Alu = mybir.AluOpType
ActF = mybir.ActivationFunctionType

B, C, H1, H2, HO = 32768, 768, 512, 256, 256
DH, LAT, K = 256, 128, 8
N_CORES = 8
TILE = 512
MAIN_TILES = 8
MAIN = MAIN_TILES * TILE  # 4096 rows per core in the single-category block

# bias_all columns (with-bias variant only)
OB1, OB2 = 0, 4
OAD1, OAD2, OAD3 = 6, 8, 10
OBD1, OBD2, OBD3 = 11, 13, 15
NBIAS = 16

nC, nH1, nH2, nDH = C // 128, H1 // 128, H2 // 128, DH // 128


def _build_nc(tail_w=32, with_bias=False, n_warm=68, n_fill=24, pads=(0, 4, 4)):
    cap = MAIN + tail_w
    nc = bacc.Bacc(name="catnet_ep")

    fT = nc.dram_tensor("fT", (C, MAIN), BF, kind="ExternalInput")
    we1 = nc.dram_tensor("we1", (C, H1), FP16, kind="ExternalInput")
    we2 = nc.dram_tensor("we2", (H1, H2), FP16, kind="ExternalInput")
    wd1a = nc.dram_tensor("wd1a", (H2, DH), FP16, kind="ExternalInput")  # We3@Wd1
    wd2a = nc.dram_tensor("wd2a", (DH, DH), FP16, kind="ExternalInput")
    wd3a = nc.dram_tensor("wd3a", (DH, LAT), FP16, kind="ExternalInput")
    if tail_w:
        fTt = nc.dram_tensor("fTt", (C, tail_w), BF, kind="ExternalInput")
        wd1b = nc.dram_tensor("wd1b", (H2, DH), FP16, kind="ExternalInput")
        wd2b = nc.dram_tensor("wd2b", (DH, DH), FP16, kind="ExternalInput")
        wd3b = nc.dram_tensor("wd3b", (DH, LAT), FP16, kind="ExternalInput")
    if with_bias:
        bias_all = nc.dram_tensor("bias_all", (128, NBIAS), F32,
                                  kind="ExternalInput")
    out = nc.dram_tensor("out", (LAT, cap), F32, kind="ExternalOutput")

    fT_r = fT.rearrange("(c p) b -> p c b", p=128)
    we1_r = we1.rearrange("(c p) h -> p c h", p=128)

    with tile.TileContext(nc) as tc:
        with (
            tc.tile_pool(name="wp", bufs=1) as wp,
            tc.tile_pool(name="fp", bufs=3) as fp,
            tc.tile_pool(name="ap", bufs=3) as ap,
            tc.tile_pool(name="dp", bufs=2) as dp,
            tc.tile_pool(name="ps_w", bufs=6, space="PSUM") as ps_w,
            tc.tile_pool(name="ps_o", bufs=2, space="PSUM") as ps_o,
        ):
            # ---- PE warm-up: dummy matmuls keep the tensor engine (and its
            # p-state ramp) running while the first weight/feature chunks
            # stream in. Inputs are a zeroed SBUF tile; the psum result is
            # never read.
            wz = wp.tile([128, 128], BF, tag="warmzero")
            nc.vector.memset(wz[:], 0.0)
            wps = ps_w.tile([128, TILE], F32, tag="pw", name="pwwarm")
            for i in range(n_warm):
                nc.tensor.matmul(wps[:, :64], wz[:, :128], wz[:, :64],
                                 start=(i == 0), stop=(i == n_warm - 1))

            def fill(n):
                # dependency-free PE work to pad unavoidable latency gaps
                if n <= 0:
                    return
                wpf = ps_w.tile([128, TILE], F32, tag="pw", name="pwfill")
                for i in range(n):
                    nc.tensor.matmul(wpf[:, :64], wz[:, :128], wz[:, :64],
                                     start=(i == 0), stop=(i == n - 1))

            # ---- startup DMAs.
            # everything latency-critical rides sync/HWDGE (~0.6us pipelined
            # issue per DMA); gpsimd/SWDGE (~1us serial descriptor-gen per
            # DMA) only carries the early out stores. we1 and fT0 parts
            # alternate on the wire and tile 0's first-layer GEMM streams
            # contraction-pair by contraction-pair right behind them.
            we1_t = wp.tile([128, nC, H1], FP16, tag="we1")
            ftb0 = fp.tile([128, nC, TILE], BF, tag="ft")
            for c0 in range(0, nC, 2):
                nc.sync.dma_start(we1_t[:, c0:c0 + 2, :], we1_r[:, c0:c0 + 2, :])
                nc.sync.dma_start(ftb0[:, c0:c0 + 2, :], fT_r[:, c0:c0 + 2, 0:TILE])
            if with_bias:
                bias_t = wp.tile([128, NBIAS], F32, tag="bias")
                nc.sync.dma_start(bias_t[:], bias_all[:])
            we2_t = wp.tile([128, nH1, H2], FP16, tag="we2")
            we2_r = we2.rearrange("(c p) h -> p c h", p=128)
            nc.sync.dma_start(we2_t[:, 0:2, :], we2_r[:, 0:2, :])
            nc.sync.dma_start(we2_t[:, 2:4, :], we2_r[:, 2:4, :])
            wd1a_t = wp.tile([128, nH2, DH], FP16, tag="wd1a")
            nc.sync.dma_start(wd1a_t[:], wd1a.rearrange("(c p) d -> p c d", p=128))
            wd2a_t = wp.tile([128, nDH, DH], FP16, tag="wd2a")
            wd3a_t = wp.tile([128, nDH, LAT], FP16, tag="wd3a")
            if tail_w:
                wd1b_t = wp.tile([128, nH2, DH], FP16, tag="wd1b")
                wd2b_t = wp.tile([128, nDH, DH], FP16, tag="wd2b")
                wd3b_t = wp.tile([128, nDH, LAT], FP16, tag="wd3b")

            # tile table: (col offset, width, weight set, moving dtype)
            tiles = [(t * TILE, TILE, "a", BF) for t in range(MAIN_TILES)]
            if tail_w:
                tiles.append((MAIN, tail_w, "b", BF))
            nt = len(tiles)

            wsets = {"a": (wd1a_t, wd2a_t, wd3a_t, OAD1, OAD2, OAD3)}
            if tail_w:
                wsets["b"] = (wd1b_t, wd2b_t, wd3b_t, OBD1, OBD2, OBD3)

            def act_relu(eng, x, pw, bcol):
                if eng == "s":
                    if with_bias:
                        nc.scalar.activation(x, pw, ActF.Relu,
                                             bias=bias_t[:, bcol:bcol + 1])
                    else:
                        nc.scalar.activation(x, pw, ActF.Relu)
                else:
                    e = nc.vector if eng == "v" else nc.gpsimd
                    if with_bias:
                        e.tensor_scalar(x, pw, bias_t[:, bcol:bcol + 1],
                                        0.0, Alu.add, Alu.max)
                    else:
                        e.tensor_scalar(x, pw, 0.0, None, Alu.max)

            def emit_enc(t, ftb):
                off, tn, _, mdt = tiles[t]
                pws = [ps_w.tile([128, TILE], F32, tag="pw", name=f"pw1_{m}")
                       for m in range(nH1)]
                a1 = []
                if t == 0:
                    # stream behind the chunk DMAs: all four psum rows
                    # advance one contraction chunk at a time (phase A),
                    # then the last two chunks are applied row-by-row
                    # (phase B) so each psum's relu dispatches early enough
                    # to cover its latency with the remaining matmuls
                    for c in range(nC - 2):
                        for m in range(nH1):
                            nc.tensor.matmul(pws[m][:, :tn],
                                             we1_t[:, c, bass.ts(m, 128)],
                                             ftb[:, c, :tn],
                                             start=(c == 0), stop=False)
                    for m in range(nH1):
                        nc.tensor.matmul(pws[m][:, :tn],
                                         we1_t[:, nC - 2, bass.ts(m, 128)],
                                         ftb[:, nC - 2, :tn],
                                         start=False, stop=False)
                        nc.tensor.matmul(pws[m][:, :tn],
                                         we1_t[:, nC - 1, bass.ts(m, 128)],
                                         ftb[:, nC - 1, :tn],
                                         start=False, stop=True)
                        x = ap.tile([128, TILE], mdt, tag=f"a1_{m}")
                        act_relu("s" if m % 2 == 0 else "v",
                                 x[:, :tn], pws[m][:, :tn], OB1 + m)
                        a1.append(x)
                else:
                    for m in range(nH1):
                        pw = pws[m]
                        for c in range(nC):
                            nc.tensor.matmul(pw[:, :tn],
                                             we1_t[:, c, bass.ts(m, 128)],
                                             ftb[:, c, :tn],
                                             start=(c == 0), stop=(c == nC - 1))
                        x = ap.tile([128, TILE], mdt, tag=f"a1_{m}")
                        act_relu("s" if m < 2 else "v",
                                 x[:, :tn], pw[:, :tn], OB1 + m)
                        a1.append(x)
                a2 = []
                if t == 0:
                    # c-outer so the first matmul only needs a1[0]
                    pw2 = [ps_w.tile([128, TILE], F32, tag="pw", name=f"pw2_{m}")
                           for m in range(nH2)]
                    for c in range(nH1):
                        for m in range(nH2):
                            nc.tensor.matmul(pw2[m][:, :tn],
                                             we2_t[:, c, bass.ts(m, 128)],
                                             a1[c][:, :tn],
                                             start=(c == 0), stop=(c == nH1 - 1))
                    for m in range(nH2):
                        x = ap.tile([128, TILE], mdt, tag=f"a2_{m}")
                        act_relu("v" if m % 2 == 0 else "s",
                                 x[:, :tn], pw2[m][:, :tn], OB2 + m)
                        a2.append(x)
                else:
                    for m in range(nH2):
                        pw = ps_w.tile([128, TILE], F32, tag="pw")
                        for c in range(nH1):
                            nc.tensor.matmul(pw[:, :tn],
                                             we2_t[:, c, bass.ts(m, 128)],
                                             a1[c][:, :tn],
                                             start=(c == 0), stop=(c == nH1 - 1))
                        x = ap.tile([128, TILE], mdt, tag=f"a2_{m}")
                        act_relu("s" if m % 2 == 0 else "v",
                                 x[:, :tn], pw[:, :tn], OB2 + m)
                        a2.append(x)
                return a2

            def emit_d1(t, h):
                off, tn, ws, mdt = tiles[t]
                wd1_t, _, _, od1, _, _ = wsets[ws]
                d1 = []
                for m in range(nDH):
                    pw = ps_w.tile([128, TILE], F32, tag="pw")
                    for c in range(nH2):
                        nc.tensor.matmul(pw[:, :tn],
                                         wd1_t[:, c, bass.ts(m, 128)],
                                         h[c][:, :tn],
                                         start=(c == 0), stop=(c == nH2 - 1))
                    # both d1 relus on the vector queue: a scalar-queue d1
                    # act (which waits on end-of-step psums) would
                    # head-of-line-block the next tile's a1 act dispatches
                    x = dp.tile([128, TILE], mdt, tag=f"d1_{m}")
                    # the last big tile's d1->d2 relu latency is exposed
                    # (no following encoder tile to hide it): run its two
                    # relus on different engines so they overlap
                    act_relu("s" if (t == nt - 2 and tail_w and m == 0) else "v",
                             x[:, :tn], pw[:, :tn], od1 + m)
                    d1.append(x)
                return d1

            def emit_d2_d3_store(t, d1, pad=False):
                off, tn, ws, mdt = tiles[t]
                _, wd2_t, wd3_t, _, od2, od3 = wsets[ws]
                if pad:
                    # skinny tail: absorb the d1->d2 relu latency with
                    # dependency-free matmuls instead of idling the PE
                    fill(pads[1])
                d2 = []
                for m in range(nDH):
                    pw = ps_w.tile([128, TILE], F32, tag="pw")
                    for c in range(nDH):
                        nc.tensor.matmul(pw[:, :tn],
                                         wd2_t[:, c, bass.ts(m, 128)],
                                         d1[c][:, :tn],
                                         start=(c == 0), stop=(c == nDH - 1))
                    x = dp.tile([128, TILE], mdt, tag=f"d2_{m}")
                    act_relu("v" if m % 2 == 0 else "s",
                             x[:, :tn], pw[:, :tn], od2 + m)
                    d2.append(x)
                if pad:
                    fill(pads[2])
                po = ps_o.tile([128, TILE], F32, tag="outps")
                for c in range(nDH):
                    nc.tensor.matmul(po[:, :tn], wd3_t[:, c, :], d2[c][:, :tn],
                                     start=(c == 0), stop=(c == nDH - 1))
                osb = ap.tile([128, TILE], F32, tag="osb")

                def copy_out(dst, src, eng):
                    if with_bias:
                        b = bias_t[:, od3:od3 + 1]
                        if eng == "v":
                            nc.vector.tensor_scalar(dst, src, b, None, Alu.add)
                        else:
                            nc.scalar.activation(dst, src, ActF.Identity, bias=b)
                    else:
                        if eng == "v":
                            nc.vector.tensor_scalar(dst, src, 0.0, None, Alu.add)
                        else:
                            nc.scalar.activation(dst, src, ActF.Identity)

                # the end of the kernel is a latency chain (copy-out act,
                # DMA issue+descgen, transfer, semaphore): split the last
                # big tile's store into halves so its transfers overlap the
                # remaining compute, keep the final (tiny tail) store on its
                # own queue, and keep everything off gpsimd, whose serial
                # descriptor-gen would add ~1us at the very end
                if t == nt - 2 and tail_w:
                    # second-to-last tile's copy-out is on the close-out
                    # critical path: halve it across both act engines
                    h2_ = tn // 2
                    copy_out(osb[:, :h2_], po[:, :h2_], "s")
                    copy_out(osb[:, h2_:tn], po[:, h2_:tn], "v")
                else:
                    copy_out(osb[:, :tn], po[:, :tn], "v")
                # last two stores on the (by then idle) non-gpsimd queues:
                # the gpsimd queue's serial descriptor-gen would add ~1us
                # at the very end of the kernel
                if t >= nt - 2:
                    eng = nc.sync
                else:
                    eng = nc.gpsimd
                eng.dma_start(out[:, off:off + tn], osb[:, :tn])

            # decoder runs one tile behind the encoder: PE order per step is
            # [enc t][d2/d3 t-1][d1 t], hiding relu latency behind
            # independent matmuls
            pend = None
            for t in range(nt):
                off, tn, _, mdt = tiles[t]
                if t == 0:
                    ftb = ftb0
                else:
                    ftb = fp.tile([128, nC, TILE if tn == TILE else tn],
                                  mdt, tag="ft" if tn == TILE else "ftt")
                    src = fT_r if tn == TILE else fTt.rearrange(
                        "(c p) b -> p c b", p=128)
                    half = nC // 2
                    so = off if tn == TILE else 0
                    nc.sync.dma_start(ftb[:, :half, :tn],
                                      src[:, :half, so:so + tn])
                    nc.sync.dma_start(ftb[:, half:, :tn],
                                      src[:, half:, so:so + tn])
                # weight DMAs ordered between the feature tiles that precede
                # their first use (all on the sync queue)
                if t == 1:
                    nc.sync.dma_start(wd2a_t[:], wd2a.rearrange("(c p) d -> p c d", p=128))
                    nc.sync.dma_start(wd3a_t[:], wd3a.rearrange("(c p) d -> p c d", p=128))
                elif t == 2 and tail_w:
                    nc.sync.dma_start(wd1b_t[:], wd1b.rearrange("(c p) d -> p c d", p=128))
                    nc.sync.dma_start(wd2b_t[:], wd2b.rearrange("(c p) d -> p c d", p=128))
                    nc.sync.dma_start(wd3b_t[:], wd3b.rearrange("(c p) d -> p c d", p=128))
                h = emit_enc(t, ftb)
                if t == 0:
                    # no previous tile's decoder to hide the relu latency
                    # between L2 and d1 — pad with dependency-free matmuls
                    fill(n_fill)
                if pend is not None:
                    emit_d2_d3_store(pend[0], pend[1])
                if tail_w and t == nt - 1:
                    # the 16-bit tail's stages are latency-dominated: pad
                    # the wait for its encoder relus with free matmuls
                    fill(pads[0])
                d1 = emit_d1(t, h)
                pend = (t, d1)
            emit_d2_d3_store(pend[0], pend[1], pad=bool(tail_w))

    nc.finalize()
    return nc


def _plan_tails(counts):
    """Assign overflow rows (beyond MAIN per category) to per-core tail
    slots: one category per core tail, tail_w rows max per core."""
    ov = {k: int(c) - MAIN for k, c in enumerate(counts) if c > MAIN}
    if not ov:
        return 0, [None] * N_CORES
    for tail_w in (32, 64, 128, 256, 384, 512):
        if sum(-(-v // tail_w) for v in ov.values()) <= N_CORES:
            break
    else:
        return None, None
    assign = []  # (cat, n_rows) per used core
    for k, v in sorted(ov.items()):
        while v > 0:
            take = min(v, tail_w)
            assign.append((k, take))
            v -= take
    assign += [None] * (N_CORES - len(assign))
    return tail_w, assign


def _chunkcols(b):
    return np.asarray(b, np.float32).reshape(-1).reshape(-1, 128).T


def _pack_inputs(features, We1, be1, We2, be2, We3, be3,
                 Wd1, bd1, Wd2, bd2, Wd3, bd3, cat_idx,
                 tail_w, tails, with_bias):
    features = np.asarray(features, np.float32)
    cat = np.asarray(cat_idx).astype(np.int64)
    order = np.argsort(cat, kind="stable")
    counts = np.bincount(cat, minlength=N_CORES)
    starts = np.zeros(N_CORES + 1, np.int64)
    np.cumsum(counts, out=starts[1:])
    cat_rows = [order[starts[k]:starts[k + 1]] for k in range(N_CORES)]

    We1f = np.asarray(We1, np.float32)
    We2f = np.asarray(We2, np.float32)
    We3f = np.asarray(We3, np.float32)
    be3f = np.asarray(be3, np.float32)
    Wd1f = np.asarray(Wd1, np.float32)
    bd1f = np.asarray(bd1, np.float32)
    Wd2f = np.asarray(Wd2, np.float32)
    bd2f = np.asarray(bd2, np.float32)
    Wd3f = np.asarray(Wd3, np.float32)
    bd3f = np.asarray(bd3, np.float32)

    we1_f16 = We1f.astype(np.float16)

    def dec_weights(k):
        wd1k = Wd1f[k]
        return (We3f @ wd1k, Wd2f[k], Wd3f[k],
                _chunkcols(wd1k.T @ be3f + bd1f[k]),
                _chunkcols(bd2f[k]), _chunkcols(bd3f[k]))

    used = {k: MAIN for k in range(N_CORES)}
    maps, row_maps = [], []
    for j in range(N_CORES):
        main_rows = cat_rows[j][:MAIN]
        f = np.zeros((MAIN, C), np.float32)
        f[:len(main_rows)] = features[main_rows]
        tail_rows = np.empty((0,), np.int64)
        tcat = j
        if tail_w and tails[j] is not None:
            tcat, n = tails[j]
            tail_rows = cat_rows[tcat][used[tcat]:used[tcat] + n]
            used[tcat] += n
        wa = dec_weights(j)
        m = {
            "fT": np.ascontiguousarray(f.T).astype(BF_NP),
            "we1": we1_f16, "we2": We2f.astype(np.float16),
            "wd1a": wa[0].astype(np.float16), "wd2a": wa[1].astype(np.float16), "wd3a": wa[2].astype(np.float16),
        }
        if tail_w:
            ft = np.zeros((tail_w, C), np.float32)
            ft[:len(tail_rows)] = features[tail_rows]
            wb = dec_weights(tcat)
            m["fTt"] = np.ascontiguousarray(ft.T).astype(BF_NP)
            m["wd1b"], m["wd2b"], m["wd3b"] = (wb[0].astype(np.float16),
                                               wb[1].astype(np.float16),
                                               wb[2].astype(np.float16))
        if with_bias:
            bias_all = np.zeros((128, NBIAS), np.float32)
            bias_all[:, OB1:OB1 + 4] = _chunkcols(be1)
            bias_all[:, OB2:OB2 + 2] = _chunkcols(be2)
            bias_all[:, OAD1:OAD1 + 2] = wa[3]
            bias_all[:, OAD2:OAD2 + 2] = wa[4]
            bias_all[:, OAD3:OAD3 + 1] = wa[5]
            if tail_w:
                bias_all[:, OBD1:OBD1 + 2] = wb[3]
                bias_all[:, OBD2:OBD2 + 2] = wb[4]
                bias_all[:, OBD3:OBD3 + 1] = wb[5]
            m["bias_all"] = bias_all
        maps.append(m)
        row_maps.append((main_rows, tail_rows))
    return maps, row_maps


_NC_CACHE = {}
_LAST_KEY = None


def _get_nc(key=None):
    global _LAST_KEY
    if key is None:
        key = _LAST_KEY if _LAST_KEY is not None else (32, False)
    if key not in _NC_CACHE:
        _NC_CACHE[key] = _build_nc(*key)
    _LAST_KEY = key
    return _NC_CACHE[key]


def kernel(**inputs) -> np.ndarray:
    cat = np.asarray(inputs["cat_idx"]).astype(np.int64)
    counts = np.bincount(cat, minlength=K)
    tail_w, tails = _plan_tails(counts)
    assert tail_w is not None, "category distribution too skewed for tails"
    with_bias = any(
        np.any(np.asarray(inputs[k], np.float32))
        for k in ("be1", "be2", "be3", "bd1", "bd2", "bd3"))
    nc = _get_nc((tail_w, with_bias))
    maps, row_maps = _pack_inputs(**inputs, tail_w=tail_w, tails=tails,
                                  with_bias=with_bias)
    res = bass_utils.run_bass_kernel_spmd(nc, maps, core_ids=list(range(N_CORES)))
    latent = np.zeros((B, LAT), np.float32)
    for j, r in enumerate(res.results):
        main_rows, tail_rows = row_maps[j]
        o = r["out"]
        latent[main_rows] = o[:, :len(main_rows)].T
        if len(tail_rows):
            latent[tail_rows] = o[:, MAIN:MAIN + len(tail_rows)].T
    return latent


# revision 52
# speedup vs baseline: 1.0004x; 1.0004x over previous
"""Trainium2 Bass kernel for nn_CategorySpecificInitNet (moe_routing).

kernel(**inputs) takes the FULL unsharded inputs (keys as in
reference.setup_inputs()) and returns the FULL [B, 128] float32 output.

Strategy — expert-parallel with overflow tails:
  - rows are dispatched to cores by category (host-side all-to-all of the
    expert-parallel alternative in the sharding hint): core k's MAIN block
    is 4096 rows (8 tiles x 512) of category k, so the whole main block
    runs ONE decoder with no routing/masking;
  - categories with more than 4096 rows spill their overflow into small
    per-core TAIL tiles (32 rows each, single category per core, possibly
    a different category than the core's main block) with a second decoder
    weight set "B". This keeps per-core work at 4096+32 rows instead of
    padding every core to the max category count — a ~6% tensor saving;
  - the encoder's third linear layer is constant-folded into each
    decoder's first layer on the host (W_f = We3 @ Wd1_k, exact algebra);
  - main tiles run with bf16 MOVING operands (features, inter-layer
    activations) against fp16 stationary weights (10 mantissa bits,
    tf32-grade; the hardware forbids mixing f32/f32r into a 16-bit
    matmul): the PE runs bf16 moving data at one row/cycle like fp32r,
    but the feature DMA halves, which is what lets tile 0's first GEMM
    start ~4.4us into the kernel streaming contraction-chunk-by-chunk
    behind the interleaved we1/features DMAs. The 32-wide tail also runs
    bf16-moving and shares the fp16 weights (encoder) / gets its own fp16
    decoder set "B";
  - relu/copy-out work is spread over the scalar and vector queues in a
    measured assignment (a1: s,s,v,v / a2: s,v / d1: v,v / d2: v,s /
    copy-out: v, with the second-to-last tile's copy-out halved across
    both act engines) — the tile scheduler coalesces activation waits
    into per-queue event semaphores, and the wrong assignment puts a
    just-in-time activation on the critical path of the next stage's
    first matmul (~117ns stall per tile); the last two output stores
    ride the sync queue so the kernel's closing store chain avoids the
    gpsimd queue's ~1us serial descriptor-gen;
  - the biases of this module are structurally zero (setup_inputs zeroes
    every bias), so the default graph variant skips bias loads and applies
    plain relu; a with-bias variant is built if any bias is nonzero;
  - compute is feature-major [features(partitions), rows(free)]; the host
    passes features pre-transposed and inverse-permutes rows on unshard;
  - per-core row tiles of 512, decoder software-pipelined one tile behind
    the encoder ([enc t][d2/d3 t-1][d1 t]);
  - a warm-up chain of dummy 64-wide matmuls keeps the tensor engine
    continuously busy from ~1us so its p-state ramp (half clock for the
    first 3us of any cold start) completes while the first DMAs are still
    in flight, and the real GEMMs run at full clock from the start.

Measured end-to-end error vs the fp32 reference: 4.0e-3 max-rel (gate 2e-2).
Measured HW exec time: 82943 ns (baseline kernel: 94286 ns).
"""
import sys

for _p in ("/opt/trn_rl_repo",):
    if _p not in sys.path:
        sys.path.append(_p)

import numpy as np
import ml_dtypes

import concourse.bass as bass
import concourse.bacc as bacc
import concourse.mybir as mybir
import concourse.tile as tile
from concourse import bass_utils

FR = mybir.dt.float32r
F32 = mybir.dt.float32
BF = mybir.dt.bfloat16
FP16 = mybir.dt.float16
BF_NP = ml_dtypes.bfloat16
Alu = mybir.AluOpType
ActF = mybir.ActivationFunctionType

B, C, H1, H2, HO = 32768, 768, 512, 256, 256
DH, LAT, K = 256, 128, 8
N_CORES = 8
TILE = 512
MAIN_TILES = 8
MAIN = MAIN_TILES * TILE  # 4096 rows per core in the single-category block

# bias_all columns (with-bias variant only)
OB1, OB2 = 0, 4
OAD1, OAD2, OAD3 = 6, 8, 10
OBD1, OBD2, OBD3 = 11, 13, 15
NBIAS = 16

nC, nH1, nH2, nDH = C // 128, H1 // 128, H2 // 128, DH // 128


def _build_nc(tail_w=32, with_bias=False, n_warm=68, n_fill=24, pads=(0, 4, 4)):
    cap = MAIN + tail_w
    nc = bacc.Bacc(name="catnet_ep")

    fT = nc.dram_tensor("fT", (C, MAIN), BF, kind="ExternalInput")
    we1 = nc.dram_tensor("we1", (C, H1), FP16, kind="ExternalInput")
    we2 = nc.dram_tensor("we2", (H1, H2), FP16, kind="ExternalInput")
    wd1a = nc.dram_tensor("wd1a", (H2, DH), FP16, kind="ExternalInput")  # We3@Wd1
    wd2a = nc.dram_tensor("wd2a", (DH, DH), FP16, kind="ExternalInput")
    wd3a = nc.dram_tensor("wd3a", (DH, LAT), FP16, kind="ExternalInput")
    if tail_w:
        fTt = nc.dram_tensor("fTt", (C, tail_w), BF, kind="ExternalInput")
        wd1b = nc.dram_tensor("wd1b", (H2, DH), FP16, kind="ExternalInput")
        wd2b = nc.dram_tensor("wd2b", (DH, DH), FP16, kind="ExternalInput")
        wd3b = nc.dram_tensor("wd3b", (DH, LAT), FP16, kind="ExternalInput")
    if with_bias:
        bias_all = nc.dram_tensor("bias_all", (128, NBIAS), F32,
                                  kind="ExternalInput")
    out = nc.dram_tensor("out", (LAT, cap), BF, kind="ExternalOutput")

    fT_r = fT.rearrange("(c p) b -> p c b", p=128)
    we1_r = we1.rearrange("(c p) h -> p c h", p=128)

    with tile.TileContext(nc) as tc:
        with (
            tc.tile_pool(name="wp", bufs=1) as wp,
            tc.tile_pool(name="fp", bufs=3) as fp,
            tc.tile_pool(name="ap", bufs=3) as ap,
            tc.tile_pool(name="dp", bufs=2) as dp,
            tc.tile_pool(name="ps_w", bufs=6, space="PSUM") as ps_w,
            tc.tile_pool(name="ps_o", bufs=2, space="PSUM") as ps_o,
        ):
            # ---- PE warm-up: dummy matmuls keep the tensor engine (and its
            # p-state ramp) running while the first weight/feature chunks
            # stream in. Inputs are a zeroed SBUF tile; the psum result is
            # never read.
            wz = wp.tile([128, 128], BF, tag="warmzero")
            nc.vector.memset(wz[:], 0.0)
            wps = ps_w.tile([128, TILE], F32, tag="pw", name="pwwarm")
            for i in range(n_warm):
                nc.tensor.matmul(wps[:, :64], wz[:, :128], wz[:, :64],
                                 start=(i == 0), stop=(i == n_warm - 1))

            def fill(n):
                # dependency-free PE work to pad unavoidable latency gaps
                if n <= 0:
                    return
                wpf = ps_w.tile([128, TILE], F32, tag="pw", name="pwfill")
                for i in range(n):
                    nc.tensor.matmul(wpf[:, :64], wz[:, :128], wz[:, :64],
                                     start=(i == 0), stop=(i == n - 1))

            # ---- startup DMAs.
            # everything latency-critical rides sync/HWDGE (~0.6us pipelined
            # issue per DMA); gpsimd/SWDGE (~1us serial descriptor-gen per
            # DMA) only carries the early out stores. we1 and fT0 parts
            # alternate on the wire and tile 0's first-layer GEMM streams
            # contraction-pair by contraction-pair right behind them.
            we1_t = wp.tile([128, nC, H1], FP16, tag="we1")
            ftb0 = fp.tile([128, nC, TILE], BF, tag="ft")
            for c0 in range(0, nC, 2):
                nc.sync.dma_start(we1_t[:, c0:c0 + 2, :], we1_r[:, c0:c0 + 2, :])
                nc.sync.dma_start(ftb0[:, c0:c0 + 2, :], fT_r[:, c0:c0 + 2, 0:TILE])
            if with_bias:
                bias_t = wp.tile([128, NBIAS], F32, tag="bias")
                nc.sync.dma_start(bias_t[:], bias_all[:])
            we2_t = wp.tile([128, nH1, H2], FP16, tag="we2")
            we2_r = we2.rearrange("(c p) h -> p c h", p=128)
            nc.sync.dma_start(we2_t[:, 0:2, :], we2_r[:, 0:2, :])
            nc.sync.dma_start(we2_t[:, 2:4, :], we2_r[:, 2:4, :])
            wd1a_t = wp.tile([128, nH2, DH], FP16, tag="wd1a")
            nc.sync.dma_start(wd1a_t[:], wd1a.rearrange("(c p) d -> p c d", p=128))
            wd2a_t = wp.tile([128, nDH, DH], FP16, tag="wd2a")
            wd3a_t = wp.tile([128, nDH, LAT], FP16, tag="wd3a")
            if tail_w:
                wd1b_t = wp.tile([128, nH2, DH], FP16, tag="wd1b")
                wd2b_t = wp.tile([128, nDH, DH], FP16, tag="wd2b")
                wd3b_t = wp.tile([128, nDH, LAT], FP16, tag="wd3b")

            # tile table: (col offset, width, weight set, moving dtype)
            tiles = [(t * TILE, TILE, "a", BF) for t in range(MAIN_TILES)]
            if tail_w:
                tiles.append((MAIN, tail_w, "b", BF))
            nt = len(tiles)

            wsets = {"a": (wd1a_t, wd2a_t, wd3a_t, OAD1, OAD2, OAD3)}
            if tail_w:
                wsets["b"] = (wd1b_t, wd2b_t, wd3b_t, OBD1, OBD2, OBD3)

            def act_relu(eng, x, pw, bcol):
                if eng == "s":
                    if with_bias:
                        nc.scalar.activation(x, pw, ActF.Relu,
                                             bias=bias_t[:, bcol:bcol + 1])
                    else:
                        nc.scalar.activation(x, pw, ActF.Relu)
                else:
                    e = nc.vector if eng == "v" else nc.gpsimd
                    if with_bias:
                        e.tensor_scalar(x, pw, bias_t[:, bcol:bcol + 1],
                                        0.0, Alu.add, Alu.max)
                    else:
                        e.tensor_scalar(x, pw, 0.0, None, Alu.max)

            def emit_enc(t, ftb):
                off, tn, _, mdt = tiles[t]
                pws = [ps_w.tile([128, TILE], F32, tag="pw", name=f"pw1_{m}")
                       for m in range(nH1)]
                a1 = []
                if t == 0:
                    # stream behind the chunk DMAs: all four psum rows
                    # advance one contraction chunk at a time (phase A),
                    # then the last two chunks are applied row-by-row
                    # (phase B) so each psum's relu dispatches early enough
                    # to cover its latency with the remaining matmuls
                    for c in range(nC - 2):
                        for m in range(nH1):
                            nc.tensor.matmul(pws[m][:, :tn],
                                             we1_t[:, c, bass.ts(m, 128)],
                                             ftb[:, c, :tn],
                                             start=(c == 0), stop=False)
                    for m in range(nH1):
                        nc.tensor.matmul(pws[m][:, :tn],
                                         we1_t[:, nC - 2, bass.ts(m, 128)],
                                         ftb[:, nC - 2, :tn],
                                         start=False, stop=False)
                        nc.tensor.matmul(pws[m][:, :tn],
                                         we1_t[:, nC - 1, bass.ts(m, 128)],
                                         ftb[:, nC - 1, :tn],
                                         start=False, stop=True)
                        x = ap.tile([128, TILE], mdt, tag=f"a1_{m}")
                        act_relu("s" if m % 2 == 0 else "v",
                                 x[:, :tn], pws[m][:, :tn], OB1 + m)
                        a1.append(x)
                else:
                    for m in range(nH1):
                        pw = pws[m]
                        for c in range(nC):
                            nc.tensor.matmul(pw[:, :tn],
                                             we1_t[:, c, bass.ts(m, 128)],
                                             ftb[:, c, :tn],
                                             start=(c == 0), stop=(c == nC - 1))
                        x = ap.tile([128, TILE], mdt, tag=f"a1_{m}")
                        act_relu("s" if m < 2 else "v",
                                 x[:, :tn], pw[:, :tn], OB1 + m)
                        a1.append(x)
                a2 = []
                if t == 0:
                    # c-outer so the first matmul only needs a1[0]
                    pw2 = [ps_w.tile([128, TILE], F32, tag="pw", name=f"pw2_{m}")
                           for m in range(nH2)]
                    for c in range(nH1):
                        for m in range(nH2):
                            nc.tensor.matmul(pw2[m][:, :tn],
                                             we2_t[:, c, bass.ts(m, 128)],
                                             a1[c][:, :tn],
                                             start=(c == 0), stop=(c == nH1 - 1))
                    for m in range(nH2):
                        x = ap.tile([128, TILE], mdt, tag=f"a2_{m}")
                        act_relu("v" if m % 2 == 0 else "s",
                                 x[:, :tn], pw2[m][:, :tn], OB2 + m)
                        a2.append(x)
                else:
                    for m in range(nH2):
                        pw = ps_w.tile([128, TILE], F32, tag="pw")
                        for c in range(nH1):
                            nc.tensor.matmul(pw[:, :tn],
                                             we2_t[:, c, bass.ts(m, 128)],
                                             a1[c][:, :tn],
                                             start=(c == 0), stop=(c == nH1 - 1))
                        x = ap.tile([128, TILE], mdt, tag=f"a2_{m}")
                        act_relu("s" if m % 2 == 0 else "v",
                                 x[:, :tn], pw[:, :tn], OB2 + m)
                        a2.append(x)
                return a2

            def emit_d1(t, h):
                off, tn, ws, mdt = tiles[t]
                wd1_t, _, _, od1, _, _ = wsets[ws]
                d1 = []
                for m in range(nDH):
                    pw = ps_w.tile([128, TILE], F32, tag="pw")
                    for c in range(nH2):
                        nc.tensor.matmul(pw[:, :tn],
                                         wd1_t[:, c, bass.ts(m, 128)],
                                         h[c][:, :tn],
                                         start=(c == 0), stop=(c == nH2 - 1))
                    # both d1 relus on the vector queue: a scalar-queue d1
                    # act (which waits on end-of-step psums) would
                    # head-of-line-block the next tile's a1 act dispatches
                    x = dp.tile([128, TILE], mdt, tag=f"d1_{m}")
                    # the last big tile's d1->d2 relu latency is exposed
                    # (no following encoder tile to hide it): run its two
                    # relus on different engines so they overlap
                    act_relu("s" if (t == nt - 2 and tail_w and m == 0) else "v",
                             x[:, :tn], pw[:, :tn], od1 + m)
                    d1.append(x)
                return d1

            def emit_d2_d3_store(t, d1, pad=False):
                off, tn, ws, mdt = tiles[t]
                _, wd2_t, wd3_t, _, od2, od3 = wsets[ws]
                if pad:
                    # skinny tail: absorb the d1->d2 relu latency with
                    # dependency-free matmuls instead of idling the PE
                    fill(pads[1])
                d2 = []
                for m in range(nDH):
                    pw = ps_w.tile([128, TILE], F32, tag="pw")
                    for c in range(nDH):
                        nc.tensor.matmul(pw[:, :tn],
                                         wd2_t[:, c, bass.ts(m, 128)],
                                         d1[c][:, :tn],
                                         start=(c == 0), stop=(c == nDH - 1))
                    x = dp.tile([128, TILE], mdt, tag=f"d2_{m}")
                    act_relu("v" if m % 2 == 0 else "s",
                             x[:, :tn], pw[:, :tn], od2 + m)
                    d2.append(x)
                if pad:
                    fill(pads[2])
                po = ps_o.tile([128, TILE], F32, tag="outps")
                for c in range(nDH):
                    nc.tensor.matmul(po[:, :tn], wd3_t[:, c, :], d2[c][:, :tn],
                                     start=(c == 0), stop=(c == nDH - 1))
                osb = ap.tile([128, TILE], BF, tag="osb")

                def copy_out(dst, src, eng):
                    if with_bias:
                        b = bias_t[:, od3:od3 + 1]
                        if eng == "v":
                            nc.vector.tensor_scalar(dst, src, b, None, Alu.add)
                        else:
                            nc.scalar.activation(dst, src, ActF.Identity, bias=b)
                    else:
                        if eng == "v":
                            nc.vector.tensor_scalar(dst, src, 0.0, None, Alu.add)
                        else:
                            nc.scalar.activation(dst, src, ActF.Identity)

                # the end of the kernel is a latency chain (copy-out act,
                # DMA issue+descgen, transfer, semaphore): split the last
                # big tile's store into halves so its transfers overlap the
                # remaining compute, keep the final (tiny tail) store on its
                # own queue, and keep everything off gpsimd, whose serial
                # descriptor-gen would add ~1us at the very end
                if t == nt - 2 and tail_w:
                    # second-to-last tile's copy-out is on the close-out
                    # critical path: halve it across both act engines
                    h2_ = tn // 2
                    copy_out(osb[:, :h2_], po[:, :h2_], "s")
                    copy_out(osb[:, h2_:tn], po[:, h2_:tn], "v")
                else:
                    copy_out(osb[:, :tn], po[:, :tn], "v")
                # last two stores on the (by then idle) non-gpsimd queues:
                # the gpsimd queue's serial descriptor-gen would add ~1us
                # at the very end of the kernel
                if t >= nt - 2:
                    eng = nc.sync
                else:
                    eng = nc.gpsimd
                eng.dma_start(out[:, off:off + tn], osb[:, :tn])

            # decoder runs one tile behind the encoder: PE order per step is
            # [enc t][d2/d3 t-1][d1 t], hiding relu latency behind
            # independent matmuls
            pend = None
            for t in range(nt):
                off, tn, _, mdt = tiles[t]
                if t == 0:
                    ftb = ftb0
                else:
                    ftb = fp.tile([128, nC, TILE if tn == TILE else tn],
                                  mdt, tag="ft" if tn == TILE else "ftt")
                    src = fT_r if tn == TILE else fTt.rearrange(
                        "(c p) b -> p c b", p=128)
                    half = nC // 2
                    so = off if tn == TILE else 0
                    nc.sync.dma_start(ftb[:, :half, :tn],
                                      src[:, :half, so:so + tn])
                    nc.sync.dma_start(ftb[:, half:, :tn],
                                      src[:, half:, so:so + tn])
                # weight DMAs ordered between the feature tiles that precede
                # their first use (all on the sync queue)
                if t == 1:
                    nc.sync.dma_start(wd2a_t[:], wd2a.rearrange("(c p) d -> p c d", p=128))
                    nc.sync.dma_start(wd3a_t[:], wd3a.rearrange("(c p) d -> p c d", p=128))
                elif t == 2 and tail_w:
                    nc.sync.dma_start(wd1b_t[:], wd1b.rearrange("(c p) d -> p c d", p=128))
                    nc.sync.dma_start(wd2b_t[:], wd2b.rearrange("(c p) d -> p c d", p=128))
                    nc.sync.dma_start(wd3b_t[:], wd3b.rearrange("(c p) d -> p c d", p=128))
                h = emit_enc(t, ftb)
                if t == 0:
                    # no previous tile's decoder to hide the relu latency
                    # between L2 and d1 — pad with dependency-free matmuls
                    fill(n_fill)
                if pend is not None:
                    emit_d2_d3_store(pend[0], pend[1])
                if tail_w and t == nt - 1:
                    # the 16-bit tail's stages are latency-dominated: pad
                    # the wait for its encoder relus with free matmuls
                    fill(pads[0])
                d1 = emit_d1(t, h)
                pend = (t, d1)
            emit_d2_d3_store(pend[0], pend[1], pad=bool(tail_w))

    nc.finalize()
    return nc


def _plan_tails(counts):
    """Assign overflow rows (beyond MAIN per category) to per-core tail
    slots: one category per core tail, tail_w rows max per core."""
    ov = {k: int(c) - MAIN for k, c in enumerate(counts) if c > MAIN}
    if not ov:
        return 0, [None] * N_CORES
    for tail_w in (32, 64, 128, 256, 384, 512):
        if sum(-(-v // tail_w) for v in ov.values()) <= N_CORES:
            break
    else:
        return None, None
    assign = []  # (cat, n_rows) per used core
    for k, v in sorted(ov.items()):
        while v > 0:
            take = min(v, tail_w)
            assign.append((k, take))
            v -= take
    assign += [None] * (N_CORES - len(assign))
    return tail_w, assign


def _chunkcols(b):
    return np.asarray(b, np.float32).reshape(-1).reshape(-1, 128).T


def _pack_inputs(features, We1, be1, We2, be2, We3, be3,
                 Wd1, bd1, Wd2, bd2, Wd3, bd3, cat_idx,
                 tail_w, tails, with_bias):
    features = np.asarray(features, np.float32)
    cat = np.asarray(cat_idx).astype(np.int64)
    order = np.argsort(cat, kind="stable")
    counts = np.bincount(cat, minlength=N_CORES)
    starts = np.zeros(N_CORES + 1, np.int64)
    np.cumsum(counts, out=starts[1:])
    cat_rows = [order[starts[k]:starts[k + 1]] for k in range(N_CORES)]

    We1f = np.asarray(We1, np.float32)
    We2f = np.asarray(We2, np.float32)
    We3f = np.asarray(We3, np.float32)
    be3f = np.asarray(be3, np.float32)
    Wd1f = np.asarray(Wd1, np.float32)
    bd1f = np.asarray(bd1, np.float32)
    Wd2f = np.asarray(Wd2, np.float32)
    bd2f = np.asarray(bd2, np.float32)
    Wd3f = np.asarray(Wd3, np.float32)
    bd3f = np.asarray(bd3, np.float32)

    we1_f16 = We1f.astype(np.float16)

    def dec_weights(k):
        wd1k = Wd1f[k]
        return (We3f @ wd1k, Wd2f[k], Wd3f[k],
                _chunkcols(wd1k.T @ be3f + bd1f[k]),
                _chunkcols(bd2f[k]), _chunkcols(bd3f[k]))

    used = {k: MAIN for k in range(N_CORES)}
    maps, row_maps = [], []
    for j in range(N_CORES):
        main_rows = cat_rows[j][:MAIN]
        f = np.zeros((MAIN, C), np.float32)
        f[:len(main_rows)] = features[main_rows]
        tail_rows = np.empty((0,), np.int64)
        tcat = j
        if tail_w and tails[j] is not None:
            tcat, n = tails[j]
            tail_rows = cat_rows[tcat][used[tcat]:used[tcat] + n]
            used[tcat] += n
        wa = dec_weights(j)
        m = {
            "fT": np.ascontiguousarray(f.T).astype(BF_NP),
            "we1": we1_f16, "we2": We2f.astype(np.float16),
            "wd1a": wa[0].astype(np.float16), "wd2a": wa[1].astype(np.float16), "wd3a": wa[2].astype(np.float16),
        }
        if tail_w:
            ft = np.zeros((tail_w, C), np.float32)
            ft[:len(tail_rows)] = features[tail_rows]
            wb = dec_weights(tcat)
            m["fTt"] = np.ascontiguousarray(ft.T).astype(BF_NP)
            m["wd1b"], m["wd2b"], m["wd3b"] = (wb[0].astype(np.float16),
                                               wb[1].astype(np.float16),
                                               wb[2].astype(np.float16))
        if with_bias:
            bias_all = np.zeros((128, NBIAS), np.float32)
            bias_all[:, OB1:OB1 + 4] = _chunkcols(be1)
            bias_all[:, OB2:OB2 + 2] = _chunkcols(be2)
            bias_all[:, OAD1:OAD1 + 2] = wa[3]
            bias_all[:, OAD2:OAD2 + 2] = wa[4]
            bias_all[:, OAD3:OAD3 + 1] = wa[5]
            if tail_w:
                bias_all[:, OBD1:OBD1 + 2] = wb[3]
                bias_all[:, OBD2:OBD2 + 2] = wb[4]
                bias_all[:, OBD3:OBD3 + 1] = wb[5]
            m["bias_all"] = bias_all
        maps.append(m)
        row_maps.append((main_rows, tail_rows))
    return maps, row_maps


_NC_CACHE = {}
_LAST_KEY = None


def _get_nc(key=None):
    global _LAST_KEY
    if key is None:
        key = _LAST_KEY if _LAST_KEY is not None else (32, False)
    if key not in _NC_CACHE:
        _NC_CACHE[key] = _build_nc(*key)
    _LAST_KEY = key
    return _NC_CACHE[key]


def kernel(**inputs) -> np.ndarray:
    cat = np.asarray(inputs["cat_idx"]).astype(np.int64)
    counts = np.bincount(cat, minlength=K)
    tail_w, tails = _plan_tails(counts)
    assert tail_w is not None, "category distribution too skewed for tails"
    with_bias = any(
        np.any(np.asarray(inputs[k], np.float32))
        for k in ("be1", "be2", "be3", "bd1", "bd2", "bd3"))
    nc = _get_nc((tail_w, with_bias))
    maps, row_maps = _pack_inputs(**inputs, tail_w=tail_w, tails=tails,
                                  with_bias=with_bias)
    res = bass_utils.run_bass_kernel_spmd(nc, maps, core_ids=list(range(N_CORES)))
    latent = np.zeros((B, LAT), np.float32)
    for j, r in enumerate(res.results):
        main_rows, tail_rows = row_maps[j]
        o = np.asarray(r["out"]).astype(np.float32)
        latent[main_rows] = o[:, :len(main_rows)].T
        if len(tail_rows):
            latent[tail_rows] = o[:, MAIN:MAIN + len(tail_rows)].T
    return latent


# revision 54
# speedup vs baseline: 1.0007x; 1.0003x over previous
"""Trainium2 Bass kernel for nn_CategorySpecificInitNet (moe_routing).

kernel(**inputs) takes the FULL unsharded inputs (keys as in
reference.setup_inputs()) and returns the FULL [B, 128] float32 output.

Strategy — expert-parallel with overflow tails:
  - rows are dispatched to cores by category (host-side all-to-all of the
    expert-parallel alternative in the sharding hint): core k's MAIN block
    is 4096 rows (8 tiles x 512) of category k, so the whole main block
    runs ONE decoder with no routing/masking;
  - categories with more than 4096 rows spill their overflow into small
    per-core TAIL tiles (32 rows each, single category per core, possibly
    a different category than the core's main block) with a second decoder
    weight set "B". This keeps per-core work at 4096+32 rows instead of
    padding every core to the max category count — a ~6% tensor saving;
  - the encoder's third linear layer is constant-folded into each
    decoder's first layer on the host (W_f = We3 @ Wd1_k, exact algebra);
  - main tiles run with bf16 MOVING operands (features, inter-layer
    activations) against fp16 stationary weights (10 mantissa bits,
    tf32-grade; the hardware forbids mixing f32/f32r into a 16-bit
    matmul): the PE runs bf16 moving data at one row/cycle like fp32r,
    but the feature DMA halves, which is what lets tile 0's first GEMM
    start ~4.4us into the kernel streaming contraction-chunk-by-chunk
    behind the interleaved we1/features DMAs. The 32-wide tail also runs
    bf16-moving and shares the fp16 weights (encoder) / gets its own fp16
    decoder set "B";
  - relu/copy-out work is spread over the scalar and vector queues in a
    measured assignment (a1: s,s,v,v / a2: s,v / d1: v,v / d2: v,s /
    copy-out: v, with the second-to-last tile's copy-out halved across
    both act engines) — the tile scheduler coalesces activation waits
    into per-queue event semaphores, and the wrong assignment puts a
    just-in-time activation on the critical path of the next stage's
    first matmul (~117ns stall per tile); the last two output stores
    ride the sync queue so the kernel's closing store chain avoids the
    gpsimd queue's ~1us serial descriptor-gen;
  - the biases of this module are structurally zero (setup_inputs zeroes
    every bias), so the default graph variant skips bias loads and applies
    plain relu; a with-bias variant is built if any bias is nonzero;
  - compute is feature-major [features(partitions), rows(free)]; the host
    passes features pre-transposed and inverse-permutes rows on unshard;
    the output rides back in bf16 (halves every store transfer, ~1e-3
    added error) and the host casts to float32;
  - per-core row tiles of 512, decoder software-pipelined one tile behind
    the encoder ([enc t][d2/d3 t-1][d1 t]);
  - a warm-up chain of dummy 64-wide matmuls keeps the tensor engine
    continuously busy from ~1us so its p-state ramp (half clock for the
    first 3us of any cold start) completes while the first DMAs are still
    in flight, and the real GEMMs run at full clock from the start.

Measured end-to-end error vs the fp32 reference: 4.8e-3 max-rel (gate 2e-2).
Measured HW exec time: 82908 ns (baseline kernel: 94286 ns).
"""
import sys

for _p in ("/opt/trn_rl_repo",):
    if _p not in sys.path:
        sys.path.append(_p)

import numpy as np
import ml_dtypes

import concourse.bass as bass
import concourse.bacc as bacc
import concourse.mybir as mybir
import concourse.tile as tile
from concourse import bass_utils

FR = mybir.dt.float32r
F32 = mybir.dt.float32
BF = mybir.dt.bfloat16
FP16 = mybir.dt.float16
BF_NP = ml_dtypes.bfloat16
Alu = mybir.AluOpType
ActF = mybir.ActivationFunctionType

B, C, H1, H2, HO = 32768, 768, 512, 256, 256
DH, LAT, K = 256, 128, 8
N_CORES = 8
TILE = 512
MAIN_TILES = 8
MAIN = MAIN_TILES * TILE  # 4096 rows per core in the single-category block

# bias_all columns (with-bias variant only)
OB1, OB2 = 0, 4
OAD1, OAD2, OAD3 = 6, 8, 10
OBD1, OBD2, OBD3 = 11, 13, 15
NBIAS = 16

nC, nH1, nH2, nDH = C // 128, H1 // 128, H2 // 128, DH // 128


def _build_nc(tail_w=32, with_bias=False, n_warm=68, n_fill=24, pads=(0, 4, 4)):
    cap = MAIN + tail_w
    nc = bacc.Bacc(name="catnet_ep")

    fT = nc.dram_tensor("fT", (C, MAIN), BF, kind="ExternalInput")
    we1 = nc.dram_tensor("we1", (C, H1), FP16, kind="ExternalInput")
    we2 = nc.dram_tensor("we2", (H1, H2), FP16, kind="ExternalInput")
    wd1a = nc.dram_tensor("wd1a", (H2, DH), FP16, kind="ExternalInput")  # We3@Wd1
    wd2a = nc.dram_tensor("wd2a", (DH, DH), FP16, kind="ExternalInput")
    wd3a = nc.dram_tensor("wd3a", (DH, LAT), FP16, kind="ExternalInput")
    if tail_w:
        fTt = nc.dram_tensor("fTt", (C, tail_w), BF, kind="ExternalInput")
        wd1b = nc.dram_tensor("wd1b", (H2, DH), FP16, kind="ExternalInput")
        wd2b = nc.dram_tensor("wd2b", (DH, DH), FP16, kind="ExternalInput")
        wd3b = nc.dram_tensor("wd3b", (DH, LAT), FP16, kind="ExternalInput")
    if with_bias:
        bias_all = nc.dram_tensor("bias_all", (128, NBIAS), F32,
                                  kind="ExternalInput")
    out = nc.dram_tensor("out", (LAT, cap), BF, kind="ExternalOutput")

    fT_r = fT.rearrange("(c p) b -> p c b", p=128)
    we1_r = we1.rearrange("(c p) h -> p c h", p=128)

    with tile.TileContext(nc) as tc:
        with (
            tc.tile_pool(name="wp", bufs=1) as wp,
            tc.tile_pool(name="fp", bufs=3) as fp,
            tc.tile_pool(name="ap", bufs=3) as ap,
            tc.tile_pool(name="dp", bufs=2) as dp,
            tc.tile_pool(name="ps_w", bufs=6, space="PSUM") as ps_w,
            tc.tile_pool(name="ps_o", bufs=2, space="PSUM") as ps_o,
        ):
            # ---- PE warm-up: dummy matmuls keep the tensor engine (and its
            # p-state ramp) running while the first weight/feature chunks
            # stream in. Inputs are a zeroed SBUF tile; the psum result is
            # never read.
            wz = wp.tile([128, 128], BF, tag="warmzero")
            nc.vector.memset(wz[:], 0.0)
            wps = ps_w.tile([128, TILE], F32, tag="pw", name="pwwarm")
            for i in range(n_warm):
                nc.tensor.matmul(wps[:, :64], wz[:, :128], wz[:, :64],
                                 start=(i == 0), stop=(i == n_warm - 1))

            def fill(n):
                # dependency-free PE work to pad unavoidable latency gaps
                if n <= 0:
                    return
                wpf = ps_w.tile([128, TILE], F32, tag="pw", name="pwfill")
                for i in range(n):
                    nc.tensor.matmul(wpf[:, :64], wz[:, :128], wz[:, :64],
                                     start=(i == 0), stop=(i == n - 1))

            # ---- startup DMAs.
            # everything latency-critical rides sync/HWDGE (~0.6us pipelined
            # issue per DMA); gpsimd/SWDGE (~1us serial descriptor-gen per
            # DMA) only carries the early out stores. we1 and fT0 parts
            # alternate on the wire and tile 0's first-layer GEMM streams
            # contraction-pair by contraction-pair right behind them.
            we1_t = wp.tile([128, nC, H1], FP16, tag="we1")
            ftb0 = fp.tile([128, nC, TILE], BF, tag="ft")
            for c0 in range(0, nC, 2):
                nc.sync.dma_start(we1_t[:, c0:c0 + 2, :], we1_r[:, c0:c0 + 2, :])
                nc.sync.dma_start(ftb0[:, c0:c0 + 2, :], fT_r[:, c0:c0 + 2, 0:TILE])
            if with_bias:
                bias_t = wp.tile([128, NBIAS], F32, tag="bias")
                nc.sync.dma_start(bias_t[:], bias_all[:])
            we2_t = wp.tile([128, nH1, H2], FP16, tag="we2")
            we2_r = we2.rearrange("(c p) h -> p c h", p=128)
            nc.sync.dma_start(we2_t[:, 0:2, :], we2_r[:, 0:2, :])
            nc.sync.dma_start(we2_t[:, 2:4, :], we2_r[:, 2:4, :])
            wd1a_t = wp.tile([128, nH2, DH], FP16, tag="wd1a")
            nc.sync.dma_start(wd1a_t[:], wd1a.rearrange("(c p) d -> p c d", p=128))
            wd2a_t = wp.tile([128, nDH, DH], FP16, tag="wd2a")
            wd3a_t = wp.tile([128, nDH, LAT], FP16, tag="wd3a")
            if tail_w:
                wd1b_t = wp.tile([128, nH2, DH], FP16, tag="wd1b")
                wd2b_t = wp.tile([128, nDH, DH], FP16, tag="wd2b")
                wd3b_t = wp.tile([128, nDH, LAT], FP16, tag="wd3b")

            # tile table: (col offset, width, weight set, moving dtype)
            tiles = [(t * TILE, TILE, "a", BF) for t in range(MAIN_TILES)]
            if tail_w:
                tiles.append((MAIN, tail_w, "b", BF))
            nt = len(tiles)

            wsets = {"a": (wd1a_t, wd2a_t, wd3a_t, OAD1, OAD2, OAD3)}
            if tail_w:
                wsets["b"] = (wd1b_t, wd2b_t, wd3b_t, OBD1, OBD2, OBD3)

            def act_relu(eng, x, pw, bcol):
                if eng == "s":
                    if with_bias:
                        nc.scalar.activation(x, pw, ActF.Relu,
                                             bias=bias_t[:, bcol:bcol + 1])
                    else:
                        nc.scalar.activation(x, pw, ActF.Relu)
                else:
                    e = nc.vector if eng == "v" else nc.gpsimd
                    if with_bias:
                        e.tensor_scalar(x, pw, bias_t[:, bcol:bcol + 1],
                                        0.0, Alu.add, Alu.max)
                    else:
                        e.tensor_scalar(x, pw, 0.0, None, Alu.max)

            def emit_enc(t, ftb):
                off, tn, _, mdt = tiles[t]
                pws = [ps_w.tile([128, TILE], F32, tag="pw", name=f"pw1_{m}")
                       for m in range(nH1)]
                a1 = []
                if t == 0:
                    # stream behind the chunk DMAs: all four psum rows
                    # advance one contraction chunk at a time (phase A),
                    # then the last two chunks are applied row-by-row
                    # (phase B) so each psum's relu dispatches early enough
                    # to cover its latency with the remaining matmuls
                    for c in range(nC - 2):
                        for m in range(nH1):
                            nc.tensor.matmul(pws[m][:, :tn],
                                             we1_t[:, c, bass.ts(m, 128)],
                                             ftb[:, c, :tn],
                                             start=(c == 0), stop=False)
                    for m in range(nH1):
                        nc.tensor.matmul(pws[m][:, :tn],
                                         we1_t[:, nC - 2, bass.ts(m, 128)],
                                         ftb[:, nC - 2, :tn],
                                         start=False, stop=False)
                        nc.tensor.matmul(pws[m][:, :tn],
                                         we1_t[:, nC - 1, bass.ts(m, 128)],
                                         ftb[:, nC - 1, :tn],
                                         start=False, stop=True)
                        x = ap.tile([128, TILE], mdt, tag=f"a1_{m}")
                        act_relu("s" if m % 2 == 0 else "v",
                                 x[:, :tn], pws[m][:, :tn], OB1 + m)
                        a1.append(x)
                else:
                    for m in range(nH1):
                        pw = pws[m]
                        for c in range(nC):
                            nc.tensor.matmul(pw[:, :tn],
                                             we1_t[:, c, bass.ts(m, 128)],
                                             ftb[:, c, :tn],
                                             start=(c == 0), stop=(c == nC - 1))
                        x = ap.tile([128, TILE], mdt, tag=f"a1_{m}")
                        act_relu("s" if m % 2 == 0 else "v",
                                 x[:, :tn], pw[:, :tn], OB1 + m)
                        a1.append(x)
                a2 = []
                if t == 0:
                    # c-outer so the first matmul only needs a1[0]
                    pw2 = [ps_w.tile([128, TILE], F32, tag="pw", name=f"pw2_{m}")
                           for m in range(nH2)]
                    for c in range(nH1):
                        for m in range(nH2):
                            nc.tensor.matmul(pw2[m][:, :tn],
                                             we2_t[:, c, bass.ts(m, 128)],
                                             a1[c][:, :tn],
                                             start=(c == 0), stop=(c == nH1 - 1))
                    for m in range(nH2):
                        x = ap.tile([128, TILE], mdt, tag=f"a2_{m}")
                        act_relu("v" if m % 2 == 0 else "s",
                                 x[:, :tn], pw2[m][:, :tn], OB2 + m)
                        a2.append(x)
                else:
                    for m in range(nH2):
                        pw = ps_w.tile([128, TILE], F32, tag="pw")
                        for c in range(nH1):
                            nc.tensor.matmul(pw[:, :tn],
                                             we2_t[:, c, bass.ts(m, 128)],
                                             a1[c][:, :tn],
                                             start=(c == 0), stop=(c == nH1 - 1))
                        x = ap.tile([128, TILE], mdt, tag=f"a2_{m}")
                        act_relu("s" if m % 2 == 0 else "v",
                                 x[:, :tn], pw[:, :tn], OB2 + m)
                        a2.append(x)
                return a2

            def emit_d1(t, h):
                off, tn, ws, mdt = tiles[t]
                wd1_t, _, _, od1, _, _ = wsets[ws]
                d1 = []
                for m in range(nDH):
                    pw = ps_w.tile([128, TILE], F32, tag="pw")
                    for c in range(nH2):
                        nc.tensor.matmul(pw[:, :tn],
                                         wd1_t[:, c, bass.ts(m, 128)],
                                         h[c][:, :tn],
                                         start=(c == 0), stop=(c == nH2 - 1))
                    # both d1 relus on the vector queue: a scalar-queue d1
                    # act (which waits on end-of-step psums) would
                    # head-of-line-block the next tile's a1 act dispatches
                    x = dp.tile([128, TILE], mdt, tag=f"d1_{m}")
                    # the last big tile's d1->d2 relu latency is exposed
                    # (no following encoder tile to hide it): run its two
                    # relus on different engines so they overlap
                    act_relu("s" if (t == nt - 2 and tail_w and m == 0) else "v",
                             x[:, :tn], pw[:, :tn], od1 + m)
                    d1.append(x)
                return d1

            def emit_d2_d3_store(t, d1, pad=False):
                off, tn, ws, mdt = tiles[t]
                _, wd2_t, wd3_t, _, od2, od3 = wsets[ws]
                if pad:
                    # skinny tail: absorb the d1->d2 relu latency with
                    # dependency-free matmuls instead of idling the PE
                    fill(pads[1])
                d2 = []
                for m in range(nDH):
                    pw = ps_w.tile([128, TILE], F32, tag="pw")
                    for c in range(nDH):
                        nc.tensor.matmul(pw[:, :tn],
                                         wd2_t[:, c, bass.ts(m, 128)],
                                         d1[c][:, :tn],
                                         start=(c == 0), stop=(c == nDH - 1))
                    x = dp.tile([128, TILE], mdt, tag=f"d2_{m}")
                    act_relu("v" if m % 2 == 0 else "s",
                             x[:, :tn], pw[:, :tn], od2 + m)
                    d2.append(x)
                if pad:
                    fill(pads[2])
                po = ps_o.tile([128, TILE], F32, tag="outps")
                for c in range(nDH):
                    nc.tensor.matmul(po[:, :tn], wd3_t[:, c, :], d2[c][:, :tn],
                                     start=(c == 0), stop=(c == nDH - 1))
                osb = ap.tile([128, TILE], BF, tag="osb")

                def copy_out(dst, src, eng):
                    if with_bias:
                        b = bias_t[:, od3:od3 + 1]
                        if eng == "v":
                            nc.vector.tensor_scalar(dst, src, b, None, Alu.add)
                        else:
                            nc.scalar.activation(dst, src, ActF.Identity, bias=b)
                    else:
                        if eng == "v":
                            nc.vector.tensor_scalar(dst, src, 0.0, None, Alu.add)
                        else:
                            nc.scalar.activation(dst, src, ActF.Identity)

                # the end of the kernel is a latency chain (copy-out act,
                # DMA issue+descgen, transfer, semaphore): split the last
                # big tile's store into halves so its transfers overlap the
                # remaining compute, keep the final (tiny tail) store on its
                # own queue, and keep everything off gpsimd, whose serial
                # descriptor-gen would add ~1us at the very end
                if t == nt - 2 and tail_w:
                    # second-to-last tile's copy-out is on the close-out
                    # critical path: halve it across both act engines
                    h2_ = tn // 2
                    copy_out(osb[:, :h2_], po[:, :h2_], "s")
                    copy_out(osb[:, h2_:tn], po[:, h2_:tn], "v")
                else:
                    copy_out(osb[:, :tn], po[:, :tn], "v")
                # last two stores on the (by then idle) non-gpsimd queues:
                # the gpsimd queue's serial descriptor-gen would add ~1us
                # at the very end of the kernel
                if t >= nt - 2:
                    eng = nc.sync
                else:
                    eng = nc.gpsimd
                eng.dma_start(out[:, off:off + tn], osb[:, :tn])

            # decoder runs one tile behind the encoder: PE order per step is
            # [enc t][d2/d3 t-1][d1 t], hiding relu latency behind
            # independent matmuls
            pend = None
            for t in range(nt):
                off, tn, _, mdt = tiles[t]
                if t == 0:
                    ftb = ftb0
                else:
                    ftb = fp.tile([128, nC, TILE if tn == TILE else tn],
                                  mdt, tag="ft" if tn == TILE else "ftt")
                    src = fT_r if tn == TILE else fTt.rearrange(
                        "(c p) b -> p c b", p=128)
                    half = nC // 2
                    so = off if tn == TILE else 0
                    nc.sync.dma_start(ftb[:, :half, :tn],
                                      src[:, :half, so:so + tn])
                    nc.sync.dma_start(ftb[:, half:, :tn],
                                      src[:, half:, so:so + tn])
                # weight DMAs ordered between the feature tiles that precede
                # their first use (all on the sync queue)
                if t == 1:
                    nc.sync.dma_start(wd2a_t[:], wd2a.rearrange("(c p) d -> p c d", p=128))
                    nc.sync.dma_start(wd3a_t[:], wd3a.rearrange("(c p) d -> p c d", p=128))
                elif t == 2 and tail_w:
                    nc.sync.dma_start(wd1b_t[:], wd1b.rearrange("(c p) d -> p c d", p=128))
                    nc.sync.dma_start(wd2b_t[:], wd2b.rearrange("(c p) d -> p c d", p=128))
                    nc.sync.dma_start(wd3b_t[:], wd3b.rearrange("(c p) d -> p c d", p=128))
                h = emit_enc(t, ftb)
                if t == 0:
                    # no previous tile's decoder to hide the relu latency
                    # between L2 and d1 — pad with dependency-free matmuls
                    fill(n_fill)
                if pend is not None:
                    emit_d2_d3_store(pend[0], pend[1])
                if tail_w and t == nt - 1:
                    # the 16-bit tail's stages are latency-dominated: pad
                    # the wait for its encoder relus with free matmuls
                    fill(pads[0])
                d1 = emit_d1(t, h)
                pend = (t, d1)
            emit_d2_d3_store(pend[0], pend[1], pad=bool(tail_w))

    nc.finalize()
    return nc


def _plan_tails(counts):
    """Assign overflow rows (beyond MAIN per category) to per-core tail
    slots: one category per core tail, tail_w rows max per core."""
    ov = {k: int(c) - MAIN for k, c in enumerate(counts) if c > MAIN}
    if not ov:
        return 0, [None] * N_CORES
    for tail_w in (32, 64, 128, 256, 384, 512):
        if sum(-(-v // tail_w) for v in ov.values()) <= N_CORES:
            break
    else:
        return None, None
    assign = []  # (cat, n_rows) per used core
    for k, v in sorted(ov.items()):
        while v > 0:
            take = min(v, tail_w)
            assign.append((k, take))
            v -= take
    assign += [None] * (N_CORES - len(assign))
    return tail_w, assign


def _chunkcols(b):
    return np.asarray(b, np.float32).reshape(-1).reshape(-1, 128).T


def _pack_inputs(features, We1, be1, We2, be2, We3, be3,
                 Wd1, bd1, Wd2, bd2, Wd3, bd3, cat_idx,
                 tail_w, tails, with_bias):
    features = np.asarray(features, np.float32)
    cat = np.asarray(cat_idx).astype(np.int64)
    order = np.argsort(cat, kind="stable")
    counts = np.bincount(cat, minlength=N_CORES)
    starts = np.zeros(N_CORES + 1, np.int64)
    np.cumsum(counts, out=starts[1:])
    cat_rows = [order[starts[k]:starts[k + 1]] for k in range(N_CORES)]

    We1f = np.asarray(We1, np.float32)
    We2f = np.asarray(We2, np.float32)
    We3f = np.asarray(We3, np.float32)
    be3f = np.asarray(be3, np.float32)
    Wd1f = np.asarray(Wd1, np.float32)
    bd1f = np.asarray(bd1, np.float32)
    Wd2f = np.asarray(Wd2, np.float32)
    bd2f = np.asarray(bd2, np.float32)
    Wd3f = np.asarray(Wd3, np.float32)
    bd3f = np.asarray(bd3, np.float32)

    we1_f16 = We1f.astype(np.float16)

    def dec_weights(k):
        wd1k = Wd1f[k]
        return (We3f @ wd1k, Wd2f[k], Wd3f[k],
                _chunkcols(wd1k.T @ be3f + bd1f[k]),
                _chunkcols(bd2f[k]), _chunkcols(bd3f[k]))

    used = {k: MAIN for k in range(N_CORES)}
    maps, row_maps = [], []
    for j in range(N_CORES):
        main_rows = cat_rows[j][:MAIN]
        f = np.zeros((MAIN, C), np.float32)
        f[:len(main_rows)] = features[main_rows]
        tail_rows = np.empty((0,), np.int64)
        tcat = j
        if tail_w and tails[j] is not None:
            tcat, n = tails[j]
            tail_rows = cat_rows[tcat][used[tcat]:used[tcat] + n]
            used[tcat] += n
        wa = dec_weights(j)
        m = {
            "fT": np.ascontiguousarray(f.T).astype(BF_NP),
            "we1": we1_f16, "we2": We2f.astype(np.float16),
            "wd1a": wa[0].astype(np.float16), "wd2a": wa[1].astype(np.float16), "wd3a": wa[2].astype(np.float16),
        }
        if tail_w:
            ft = np.zeros((tail_w, C), np.float32)
            ft[:len(tail_rows)] = features[tail_rows]
            wb = dec_weights(tcat)
            m["fTt"] = np.ascontiguousarray(ft.T).astype(BF_NP)
            m["wd1b"], m["wd2b"], m["wd3b"] = (wb[0].astype(np.float16),
                                               wb[1].astype(np.float16),
                                               wb[2].astype(np.float16))
        if with_bias:
            bias_all = np.zeros((128, NBIAS), np.float32)
            bias_all[:, OB1:OB1 + 4] = _chunkcols(be1)
            bias_all[:, OB2:OB2 + 2] = _chunkcols(be2)
            bias_all[:, OAD1:OAD1 + 2] = wa[3]
            bias_all[:, OAD2:OAD2 + 2] = wa[4]
            bias_all[:, OAD3:OAD3 + 1] = wa[5]
            if tail_w:
                bias_all[:, OBD1:OBD1 + 2] = wb[3]
                bias_all[:, OBD2:OBD2 + 2] = wb[4]
                bias_all[:, OBD3:OBD3 + 1] = wb[5]
            m["bias_all"] = bias_all
        maps.append(m)
        row_maps.append((main_rows, tail_rows))
    return maps, row_maps


_NC_CACHE = {}
_LAST_KEY = None


def _get_nc(key=None):
    global _LAST_KEY
    if key is None:
        key = _LAST_KEY if _LAST_KEY is not None else (32, False)
    if key not in _NC_CACHE:
        _NC_CACHE[key] = _build_nc(*key)
    _LAST_KEY = key
    return _NC_CACHE[key]


def kernel(**inputs) -> np.ndarray:
    cat = np.asarray(inputs["cat_idx"]).astype(np.int64)
    counts = np.bincount(cat, minlength=K)
    tail_w, tails = _plan_tails(counts)
    assert tail_w is not None, "category distribution too skewed for tails"
    with_bias = any(
        np.any(np.asarray(inputs[k], np.float32))
        for k in ("be1", "be2", "be3", "bd1", "bd2", "bd3"))
    nc = _get_nc((tail_w, with_bias))
    maps, row_maps = _pack_inputs(**inputs, tail_w=tail_w, tails=tails,
                                  with_bias=with_bias)
    res = bass_utils.run_bass_kernel_spmd(nc, maps, core_ids=list(range(N_CORES)))
    latent = np.zeros((B, LAT), np.float32)
    for j, r in enumerate(res.results):
        main_rows, tail_rows = row_maps[j]
        o = np.asarray(r["out"]).astype(np.float32)
        latent[main_rows] = o[:, :len(main_rows)].T
        if len(tail_rows):
            latent[tail_rows] = o[:, MAIN:MAIN + len(tail_rows)].T
    return latent
